# revision 1
# baseline (speedup 1.0000x reference)
"""Dense CRF forward (5 mean-field iterations, exact dense bilateral kernel)
on 8 Trainium2 NeuronCores via Bass/Tile.

Sharding: core c -> (batch n = c//4, block j = c%4). Each core owns 1024 of the
4096 pixels of one batch element: it builds and keeps resident in SBUF the
[4096 x 1024] column-block W of the (symmetric) matrix 4*K, where
K[p,q] = exp(-0.5*||f_p - f_q||^2). Each iteration computes
  q_hat[p,c] = U[p,c] + sum_q W[q,p]*q[q,c] + 2*(spatial conv)(q)[p,c]
then softmax over c, then all-gathers the new q among the 4 cores of the batch
group. The spatial 71x71 depthwise Gaussian conv is exactly separable (gk is
rank-1), computed as two 64x64 Toeplitz-matrix matmul stages built from gk.

The bilateral exponent is computed as a single-pass float32r matmul over
hi/lo-split augmented features chosen so every operand is exactly
representable in fp32r's 11-bit mantissa (error ~3e-4 on the exponent); the
exp runs on the scalar engine writing W in float32r so the per-iteration
matmuls run at full PE rate (1 cycle/row).
"""
import os
import sys

for _p in ("/opt/trn_rl_repo", "/root/.axon_site/_ro/trn_rl_repo"):
    if os.path.isdir(_p) and _p not in sys.path:
        sys.path.insert(0, _p)

import numpy as np
import concourse.bass as bass  # noqa: E402
import concourse.tile as tile  # noqa: E402
from concourse import mybir, bacc  # noqa: E402
from concourse.bass_utils import run_bass_kernel_spmd  # noqa: E402

F32 = mybir.dt.float32
F32R = mybir.dt.float32 if os.environ.get("KERNEL_F32") else mybir.dt.float32r
EXP = mybir.ActivationFunctionType.Exp
AX = mybir.AxisListType
ALU = mybir.AluOpType

N, C, H, W_IMG = 2, 21, 64, 64
P = H * W_IMG            # 4096 pixels
NB = 4                   # blocks (cores) per batch element
PB = P // NB             # 1024 pixels per block
T = P // 128             # 32 q-tiles of 128 pixels
PC = PB // 128           # 8 p-chunks of 128 pixels per block
NUM_ITER = 5
COMPAT_BF, COMPAT_SPATIAL = 4.0, 2.0
KD = 18                  # split-feature contraction dims

TRACE = False
LAST_EXEC_NS = None
LAST_RESULTS = None

_CACHED_NC = None


def _build_program():
    nc = bacc.Bacc("TRN2", target_bir_lowering=False, debug=False, num_devices=8)

    fA_d = nc.dram_tensor("fa", [KD, P], F32, kind="ExternalInput")
    fB_d = nc.dram_tensor("fb", [KD, PB], F32, kind="ExternalInput")
    u_d = nc.dram_tensor("u_blk", [128, PC * C], F32, kind="ExternalInput")
    q0_d = nc.dram_tensor("q0cc", [NB * 128, PC * C], F32, kind="ExternalInput")
    am_d = nc.dram_tensor("amat", [64, 64], F32, kind="ExternalInput")
    a2_d = nc.dram_tensor("a2mat", [64, 64], F32, kind="ExternalInput")
    ay_d = nc.dram_tensor("ay", [64, 16], F32, kind="ExternalInput")
    out_d = nc.dram_tensor("out_blk", [128, PC * C], F32, kind="ExternalOutput")
    id_d = nc.inline_tensor(np.eye(128, dtype=np.float32), name="ident_np")

    with tile.TileContext(nc) as tc:
        with (
            tc.tile_pool(name="const", bufs=1) as cpool,
            tc.tile_pool(name="wpool", bufs=1) as wpool,
            tc.tile_pool(name="qpool", bufs=1) as qpool,
            tc.tile_pool(name="work", bufs=1) as work,
            tc.tile_pool(name="ps_build", bufs=2, space="PSUM") as ps_build,
            tc.tile_pool(name="ps_qbf", bufs=2, space="PSUM") as ps_qbf,
            tc.tile_pool(name="ps_bt", bufs=1, space="PSUM") as ps_bt,
            tc.tile_pool(name="ps_conv", bufs=1, space="PSUM") as ps_conv,
            tc.tile_pool(name="dram", bufs=1, space="DRAM") as dram,
        ):
            # ---- constants / inputs to SBUF ----
            fa_raw = cpool.tile([KD, P], F32, tag="fa_raw")
            fb_raw = cpool.tile([KD, PB], F32, tag="fb_raw")
            fa = cpool.tile([KD, P], F32R, tag="fa")
            fb = cpool.tile([KD, PB], F32R, tag="fb")
            u_t = cpool.tile([128, PC * C], F32, tag="u_t")
            am_raw = cpool.tile([64, 64], F32, tag="am_raw")
            a2_raw = cpool.tile([64, 64], F32, tag="a2_raw")
            ay_raw = cpool.tile([64, 16], F32, tag="ay_raw")
            a2 = cpool.tile([64, 64], F32R, tag="a2")
            ay = cpool.tile([64, 16], F32R, tag="ay")
            ident = cpool.tile([128, 128], F32, tag="ident")
            nc.sync.dma_start(fa_raw[:], fA_d[:])
            nc.sync.dma_start(fb_raw[:], fB_d[:])
            nc.sync.dma_start(u_t[:], u_d[:])
            nc.sync.dma_start(am_raw[:], am_d[:])
            nc.sync.dma_start(a2_raw[:], a2_d[:])
            nc.sync.dma_start(ay_raw[:], ay_d[:])
            nc.sync.dma_start(ident[:], id_d[:])
            nc.scalar.copy(fa[:], fa_raw[:])
            nc.scalar.copy(fb[:], fb_raw[:])
            nc.scalar.copy(a2[:], a2_raw[:])
            nc.scalar.copy(ay[:], ay_raw[:])

            # ---- W build: W[q, p] = exp(fa_q . fb_p), fp32r matmul + ACT exp ----
            w_sb = wpool.tile([128, T * PB], F32R, tag="wsb")
            for t in range(T):
                pb = ps_build.tile([128, 1024], F32, tag="pbuild")
                for h in range(2):
                    nc.tensor.matmul(
                        pb[:, h * 512:(h + 1) * 512],
                        fa[:, t * 128:(t + 1) * 128], fb[:, h * 512:(h + 1) * 512],
                        start=True, stop=True)
                nc.scalar.activation(
                    w_sb[:, t * PB:(t + 1) * PB], pb[:], EXP, bias=0.0, scale=1.0)

            # ---- persistent working tiles ----
            qpc_raw = qpool.tile([128, T * C], F32, tag="qpc_raw")
            qpc_r = qpool.tile([128, T * C], F32R, tag="qpc_r")
            qyl = qpool.tile([64, 64 * C], F32, tag="qyl")
            qyl_r = qpool.tile([64, 64 * C], F32R, tag="qyl_r")
            s_qbf = work.tile([C, PB], F32, tag="s_qbf")
            s_t2 = work.tile([64, C * 16], F32R, tag="s_t2")
            t0 = work.tile([128, PC * C], F32, tag="t0")
            e_t = work.tile([128, PC * C], F32, tag="e_t")
            negm = work.tile([128, PC], F32, tag="negm")
            ssum = work.tile([128, PC], F32, tag="ssum")
            rsum = work.tile([128, PC], F32, tag="rsum")
            qstage = work.tile([128, PC * C], F32, tag="qstage")

            def load_q(src_ap):
                """src_ap: DRAM tensor [512, 168] in cc layout:
                offset = r*21504 + part*168 + pcl*21 + c ; pixel = r*1024+pcl*128+part.
                Loads qpc_raw [128, (t=(r,pcl), c)] and qyl [64 (y'), (x', c)]."""
                src_pc = src_ap[:].rearrange(
                    "(r part) (pcl c) -> part r pcl c", r=NB, c=C)
                dst_pc = qpc_raw[:].rearrange(
                    "part (r pcl c) -> part r pcl c", r=NB, pcl=PC)
                nc.sync.dma_start(dst_pc, src_pc)
                # y' = 16*ya + 2*yb + ylo ; part = ylo*64 + x'
                src_y = src_ap[:].rearrange(
                    "(ya ylo xp) (yb c) -> ya yb ylo xp c", ya=NB, ylo=2, c=C)
                for ya in range(NB):
                    for yb in range(PC):
                        nc.gpsimd.dma_start(
                            qyl[ya * 16 + yb * 2: ya * 16 + yb * 2 + 2, :]
                            .rearrange("ylo (xp c) -> ylo xp c", c=C),
                            src_y[ya, yb])

            load_q(q0_d)

            for it in range(NUM_ITER):
                # fp32r copies for the PE (must be produced by a compute op)
                nc.vector.tensor_copy(qpc_r[:], qpc_raw[:])
                nc.vector.tensor_copy(qyl_r[:], qyl[:])

                # ---- qbf: out[c, p] = sum_q W[q, p] * q[q, c] ----
                for h in range(2):
                    pq = ps_qbf.tile([C, 512], F32, tag="pqbf")
                    for t in range(T):
                        nc.tensor.matmul(
                            pq[:], qpc_r[:, t * C:(t + 1) * C],
                            w_sb[:, t * PB + h * 512: t * PB + (h + 1) * 512],
                            start=(t == 0), stop=(t == T - 1))
                    nc.scalar.copy(s_qbf[:, h * 512:(h + 1) * 512], pq[:])

                # transpose to [p, c] chunks
                pbt = ps_bt.tile([128, PC * C], F32, tag="pbt")
                for pc in range(PC):
                    nc.tensor.transpose(
                        pbt[:, pc * C:(pc + 1) * C],
                        s_qbf[:, pc * 128:(pc + 1) * 128], ident[:C, :C])

                # ---- spatial conv (exact separable): T2 then T3 ----
                pt2 = ps_conv.tile([64, C * 16], F32, tag="pconv")
                qyl_v = qyl_r[:].rearrange("p (x c) -> p c x", c=C)
                for ci in range(C):
                    nc.tensor.matmul(pt2[:, ci * 16:(ci + 1) * 16],
                                     qyl_v[:, ci, :], ay[:],
                                     start=True, stop=True)
                nc.scalar.copy(s_t2[:], pt2[:])
                pt3 = ps_conv.tile([64, C * 16], F32, tag="pconv")
                nc.tensor.matmul(pt3[:], a2[:], s_t2[:], start=True, stop=True)

                # ---- epilogue: t0 = U + qbf + qsf; softmax over c ----
                nc.vector.tensor_tensor(t0[:], pbt[:], u_t[:], op=ALU.add)
                t3v = pt3[:].rearrange("p (c pc ylo) -> p ylo pc c", pc=PC, ylo=2)
                for ylo in range(2):
                    dst = t0[ylo * 64:(ylo + 1) * 64, :].rearrange(
                        "p (pc c) -> p pc c", c=C)
                    nc.vector.tensor_tensor(dst, dst, t3v[:, ylo], op=ALU.add)
                t0v = t0[:].rearrange("p (pc c) -> p pc c", c=C)
                nc.vector.tensor_reduce(negm[:], t0v, axis=AX.X, op=ALU.max,
                                        negate=True)
                for pc in range(PC):
                    nc.scalar.activation(
                        e_t[:, pc * C:(pc + 1) * C], t0[:, pc * C:(pc + 1) * C],
                        EXP, bias=negm[:, pc:pc + 1], scale=1.0)
                nc.vector.tensor_reduce(
                    ssum[:], e_t[:].rearrange("p (pc c) -> p pc c", c=C),
                    axis=AX.X, op=ALU.add)
                nc.vector.reciprocal(rsum[:], ssum[:])
                for pc in range(PC):
                    nc.vector.tensor_scalar_mul(
                        qstage[:, pc * C:(pc + 1) * C],
                        e_t[:, pc * C:(pc + 1) * C], rsum[:, pc:pc + 1])

                if it < NUM_ITER - 1:
                    cc_in = nc.dram_tensor(f"ccin{it}", [128, PC * C], F32,
                                           kind="Internal")
                    cc_out = nc.dram_tensor(f"ccout{it}", [NB * 128, PC * C], F32,
                                            kind="Internal")
                    nc.sync.dma_start(cc_in[:], qstage[:])
                    nc.gpsimd.collective_compute(
                        "AllGather", ALU.bypass,
                        replica_groups=[[0, 1, 2, 3], [4, 5, 6, 7]],
                        ins=[cc_in[:]], outs=[cc_out[:]])
                    load_q(cc_out)
                else:
                    nc.sync.dma_start(out_d[:], qstage[:])

    nc.compile()
    return nc


def _host_inputs(unary, ref, gk, kstd):
    """Build the 8 per-core input maps (fp64 host math, fp32 cast)."""
    unary = np.asarray(unary, np.float64)
    ref = np.asarray(ref, np.float64)
    gk = np.asarray(gk, np.float64)
    kstd = np.asarray(kstd, np.float64)

    yy, xx = np.meshgrid(np.arange(H, dtype=np.float64),
                         np.arange(W_IMG, dtype=np.float64), indexing="ij")
    grid = np.broadcast_to(np.stack([yy, xx])[None], (N, 2, H, W_IMG))
    stacked = np.concatenate([grid, ref], axis=1)
    feats = (stacked / kstd[None, :, None, None]).reshape(N, 5, P)  # [N,5,P]

    # hi/lo split so every matmul operand is exact in fp32r's 11-bit mantissa
    ctr = np.array([31.5 / kstd[0], 31.5 / kstd[1],
                    127.5 / kstd[2], 127.5 / kstd[3], 127.5 / kstd[4]])
    fc = feats - ctr[None, :, None]
    fs = np.round(fc[:, :2] * 8192) / 8192          # spatial, exact on 2^-13 grid
    hh = np.round(fc[:, 2:] * 64) / 64              # color hi, exact on 2^-6 grid
    ll = fc[:, 2:] - hh                             # color lo (|l| <= 2^-7)
    Feff = np.concatenate([fs, hh + ll], axis=1)
    sq = np.sum(Feff * Feff, axis=1)                # [N,P]
    ln4 = np.log(COMPAT_BF)

    U = np.log(np.clip(unary, 1e-5, 1.0)).reshape(N, C, P)
    q0 = np.exp(U - U.max(axis=1, keepdims=True))
    q0 = q0 / q0.sum(axis=1, keepdims=True)

    g2 = gk[0, 0]
    v = g2[:, 35] / np.sqrt(g2[35, 35])
    A = np.zeros((64, 64), np.float64)
    for a in range(64):
        for b in range(64):
            if abs(b - a) <= 35:
                A[a, b] = v[b - a + 35]

    in_maps = []
    for core in range(8):
        n, j = core // NB, core % NB
        blk = slice(j * PB, (j + 1) * PB)
        one = np.ones(P)
        Hq = np.round(-0.5 * sq[n] * 8) / 8
        Lq = -0.5 * sq[n] - Hq
        Hp = np.round((-0.5 * sq[n] + ln4) * 8) / 8
        Lp = (-0.5 * sq[n] + ln4) - Hp
        a_dims = [fs[n][0], fs[n][1]]
        b_dims = [fs[n][0], fs[n][1]]
        for ci in range(3):
            a_dims += [hh[n][ci], hh[n][ci], ll[n][ci], ll[n][ci]]
            b_dims += [hh[n][ci], ll[n][ci], hh[n][ci], ll[n][ci]]
        a_dims += [Hq, Lq, one, one]
        b_dims += [one, one, Hp, Lp]
        fa = np.stack(a_dims).astype(np.float32)            # [18, P]
        fb = np.stack(b_dims)[:, blk].astype(np.float32)    # [18, PB]
        u_blk = (U[n].T[blk]
                 .reshape(PC, 128, C).transpose(1, 0, 2)
                 .reshape(128, PC * C).astype(np.float32))
        q0cc = (q0[n].T
                .reshape(NB, PC, 128, C).transpose(0, 2, 1, 3)
                .reshape(NB * 128, PC * C).astype(np.float32))
        in_maps.append({
            "fa": fa, "fb": fb, "u_blk": u_blk, "q0cc": q0cc,
            "amat": A.astype(np.float32),
            "a2mat": (COMPAT_SPATIAL * A).astype(np.float32),
            "ay": A[:, j * 16:(j + 1) * 16].astype(np.float32),
        })
    return in_maps


def kernel(unary, ref, gk, kstd):
    global _CACHED_NC, LAST_EXEC_NS, LAST_RESULTS
    in_maps = _host_inputs(unary, ref, gk, kstd)
    if _CACHED_NC is None:
        _CACHED_NC = _build_program()
    res = run_bass_kernel_spmd(_CACHED_NC, in_maps, core_ids=list(range(8)),
                               trace=TRACE)
    LAST_EXEC_NS = res.exec_time_ns
    LAST_RESULTS = res
    q_full = np.zeros((N, P, C), np.float32)
    for core in range(8):
        n, j = core // NB, core % NB
        blk = res.results[core]["out_blk"]
        q_full[n, j * PB:(j + 1) * PB] = (
            blk.reshape(128, PC, C).transpose(1, 0, 2).reshape(PB, C))
    return q_full.transpose(0, 2, 1).reshape(N, C, H, W_IMG).astype(np.float32)

